# revision 1
# baseline (speedup 1.0000x reference)
"""Trainium2 Bass kernel for nn_DfOpCoefLoop (deep-filter complex FIR + alpha blend).

Reference semantics (per batch b, time t, freq bin f < 96):
    spec_f[t,f] = sum_{i=0..4} x[t+i-2, f] * coefs[t,i,f]      (complex MAC, zero-padded in t)
    out[t,f]    = alpha[t] * spec_f[t,f] + (1-alpha[t]) * x[t,f]
    out[t,f]    = spec[t,f]                                    (f >= 96 passthrough)

Strategy: pure data-parallel over batch (32 batches -> 8 cores x 4 batches).

The host pre-arranges every input in the exact order the engines consume it, so
the device program is ~40 large DMAs plus a few hundred contiguous compute ops:
  - X5: the 5 time-shifted windows, deinterleaved and stacked per t-row as
    [xr taps (5,96) | xi taps (5,96)], zero-padded at the t edges, then blocked
    to (partition = t%128, chunk = t//128).
  - CX: coefs per t-row as [cr (5,96) | -ci (5,96)] (ci pre-negated), blocked
    the same way.
  - alpha / (1-alpha) as per-partition scalar tables [128, batch*chunk].
Compute runs in 4-chunk groups (tensor_reduce has ~1us fixed cost, so reduces
are merged across chunks):
    m1 = X5*CX = [xr*cr | xi*(-ci)]   --one 10-tap reduce--> re     (DVE)
    m2a = xi*cr (TT), m2b = (xr*-1)*(-ci) = xr*ci (STT imm)         (GPSIMD)
                                      --one 10-tap reduce--> im     (DVE)
    out = alpha*(re|im) + (1-alpha)*x0   (per-chunk STT; (1-a)*x0 on ScalarE)
The f>=96 bins never touch the device: the host copies them straight from the
input when assembling the full output (identity passthrough).
"""

import numpy as np

ORDER = 5
LOOKAHEAD = 2
F = 96            # deep-filtered bins
FC = 2 * F        # one t-row of (c,f) planar data: 192 floats
HB = ORDER * F    # 480: one comp block of taps
W = ORDER * FC    # 960: stacked taps [xr5 | xi5] / coef row [cr5 | mci5]
NFREQ = 481
B, T = 32, 1000
NCORES = 8
BPC = B // NCORES  # batches per core
GRP = 4            # chunks per compute group

_CACHE = {}


def _build_program(bpc, t_len):
    """Build the per-core Bass program (returns a compiled Bacc)."""
    import concourse.bacc as bacc
    import concourse.mybir as mybir
    import concourse.tile as tile

    nk = (t_len + 127) // 128          # time chunks per batch
    assert nk % GRP == 0
    ncols = bpc * nk                   # alpha table columns
    GW = GRP * W                       # group free-dim span (3840)

    # Bacc (not raw Bass): its compile() runs generate_event_semaphores,
    # which splits multi-wait sync onto EventSemaphore instructions --
    # TRN2 instructions encode at most one sem wait.
    nc = bacc.Bacc("TRN2", target_bir_lowering=False, debug=False)
    dt = mybir.dt.float32

    x5_t = nc.dram_tensor("x5_t", [bpc, 128, nk * W], dt, kind="ExternalInput").ap()
    cx_t = nc.dram_tensor("cx_t", [bpc, 128, nk * W], dt, kind="ExternalInput").ap()
    alpha_t = nc.dram_tensor("alpha_t", [128, ncols], dt, kind="ExternalInput").ap()
    oma_t = nc.dram_tensor("oma_t", [128, ncols], dt, kind="ExternalInput").ap()
    outb = nc.dram_tensor("outb", [bpc, 128, nk * FC], dt, kind="ExternalOutput").ap()

    mul = mybir.AluOpType.mult
    add = mybir.AluOpType.add
    copy_fn = mybir.ActivationFunctionType.Copy

    def jview(t, off, run):
        """(j, run)-strided view of a group tile: j-stride W, GRP rows."""
        return t.rearrange("p (j w) -> p j w", j=GRP, w=W)[:, :, off : off + run]

    with tile.TileContext(nc) as tc:
        with (
            tc.tile_pool(name="const", bufs=1) as const_pool,
            tc.tile_pool(name="x5g", bufs=2) as x5_pool,
            tc.tile_pool(name="cxg", bufs=2) as cx_pool,
            tc.tile_pool(name="obp", bufs=2) as ob_pool,
            tc.tile_pool(name="p1", bufs=2) as p1_pool,
            tc.tile_pool(name="p2", bufs=2) as p2_pool,
            tc.tile_pool(name="small", bufs=3) as small_pool,
        ):
            alpha_sb = const_pool.tile([128, ncols], dt, name="alpha_sb")
            oma_sb = const_pool.tile([128, ncols], dt, name="oma_sb")
            nc.sync.dma_start(alpha_sb[:], alpha_t[:])
            nc.sync.dma_start(oma_sb[:], oma_t[:])

            for b in range(bpc):
                ob = ob_pool.tile([128, nk * FC], dt, name="ob")
                for g in range(nk // GRP):
                    gs = g * GW
                    x5g = x5_pool.tile([128, GW], dt, name="x5g")
                    cxg = cx_pool.tile([128, GW], dt, name="cxg")
                    nc.sync.dma_start(x5g[:], x5_t[b, :, gs : gs + GW])
                    nc.scalar.dma_start(cxg[:], cx_t[b, :, gs : gs + GW])

                    p1 = p1_pool.tile([128, GW], dt, name="p1")
                    p2 = p2_pool.tile([128, GW], dt, name="p2")
                    acc = small_pool.tile([128, GRP * FC], dt, name="acc")
                    v = small_pool.tile([128, GRP * FC], dt, name="v")

                    # m1 = X5 * CX (fully contiguous); alternate engine
                    m1_eng = nc.vector if (b + g) % 2 == 0 else nc.gpsimd
                    m1_eng.tensor_mul(p1[:], x5g[:], cxg[:])
                    # re = sum of the 10 (comp,tap) products
                    nc.vector.tensor_reduce(
                        acc[:].rearrange("p (j c f) -> p j c f", j=GRP, c=2, f=F)[
                            :, :, 0:1
                        ].squeeze(2),
                        p1[:].rearrange(
                            "p (j gi f) -> p j f gi", j=GRP, gi=2 * ORDER, f=F
                        ),
                        axis=mybir.AxisListType.X,
                        op=add,
                    )
                    # m2a = xi*cr (GPSIMD) ; m2b = (xr*-1)*(-ci) = xr*ci (DVE STT)
                    nc.gpsimd.tensor_mul(
                        jview(p2, 0, HB), jview(x5g, HB, HB), jview(cxg, 0, HB)
                    )
                    nc.vector.scalar_tensor_tensor(
                        jview(p2, HB, HB),
                        jview(x5g, 0, HB),
                        -1.0,
                        jview(cxg, HB, HB),
                        op0=mul,
                        op1=mul,
                    )
                    # im = sum of the 10 products
                    nc.vector.tensor_reduce(
                        acc[:].rearrange("p (j c f) -> p j c f", j=GRP, c=2, f=F)[
                            :, :, 1:2
                        ].squeeze(2),
                        p2[:].rearrange(
                            "p (j gi f) -> p j f gi", j=GRP, gi=2 * ORDER, f=F
                        ),
                        axis=mybir.AxisListType.X,
                        op=add,
                    )
                    # blend per chunk (alpha is a per-(b,chunk) partition scalar)
                    for kk in range(GRP):
                        col = b * nk + g * GRP + kk
                        # v = (1-alpha) * x0 ; x0 = tap d=0 of X5 (planar view)
                        nc.scalar.activation(
                            v[:, kk * FC : (kk + 1) * FC].rearrange(
                                "p (c f) -> p c f", c=2, f=F
                            ),
                            x5g[:, kk * W : (kk + 1) * W]
                            .rearrange("p (c i f) -> p c i f", c=2, i=ORDER, f=F)[
                                :, :, LOOKAHEAD : LOOKAHEAD + 1
                            ]
                            .squeeze(2),
                            copy_fn,
                            scale=oma_sb[:, col : col + 1],
                        )
                        # out = alpha*acc + v
                        nc.vector.scalar_tensor_tensor(
                            ob[:, (g * GRP + kk) * FC : (g * GRP + kk + 1) * FC],
                            acc[:, kk * FC : (kk + 1) * FC],
                            alpha_sb[:, col : col + 1],
                            v[:, kk * FC : (kk + 1) * FC],
                            op0=mul,
                            op1=add,
                        )

                nc.sync.dma_start(outb[b], ob[:])
    nc.compile()
    return nc


def _get_program(bpc=BPC, t_len=T):
    key = (bpc, t_len)
    if key not in _CACHE:
        _CACHE[key] = _build_program(bpc, t_len)
    return _CACHE[key]


def _block(a, nk):
    """(nk*128, R) -> [128, nk*R] with partition = t%128."""
    n, r = a.shape
    assert n == nk * 128
    return np.ascontiguousarray(
        a.reshape(nk, 128, r).transpose(1, 0, 2).reshape(128, nk * r)
    )


def _host_prep(spec, coefs, alpha, bpc, t_len):
    """Re-layout one core's inputs into the device consumption order."""
    nk = (t_len + 127) // 128
    tp = nk * 128
    spec2 = np.asarray(spec[:, 0], dtype=np.float32)          # (bpc, t, 481, 2)
    xr = spec2[:, :, :F, 0]                                    # (bpc, t, 96)
    xi = spec2[:, :, :F, 1]
    xrp = np.zeros((bpc, tp + ORDER - 1, F), np.float32)
    xip = np.zeros((bpc, tp + ORDER - 1, F), np.float32)
    xrp[:, LOOKAHEAD : LOOKAHEAD + t_len] = xr
    xip[:, LOOKAHEAD : LOOKAHEAD + t_len] = xi
    # taps: X5[t, i, f] = x[t + i - LOOKAHEAD]
    xr5 = np.stack([xrp[:, i : i + tp] for i in range(ORDER)], axis=2)  # (bpc,tp,5,96)
    xi5 = np.stack([xip[:, i : i + tp] for i in range(ORDER)], axis=2)
    x5 = np.concatenate(
        [xr5.reshape(bpc, tp, HB), xi5.reshape(bpc, tp, HB)], axis=2
    )                                                          # (bpc, tp, 960)

    cr = np.asarray(coefs[..., 0], dtype=np.float32).reshape(bpc, t_len, HB)
    ci = np.asarray(coefs[..., 1], dtype=np.float32).reshape(bpc, t_len, HB)
    cx = np.zeros((bpc, tp, W), np.float32)
    cx[:, :t_len, :HB] = cr
    cx[:, :t_len, HB:] = -ci

    x5_t = np.stack([_block(x5[b], nk) for b in range(bpc)])
    cx_t = np.stack([_block(cx[b], nk) for b in range(bpc)])

    al = np.zeros((bpc, tp), np.float32)
    al[:, :t_len] = alpha[:, :, 0]
    alpha_t = np.ascontiguousarray(
        al.reshape(bpc, nk, 128).transpose(2, 0, 1).reshape(128, bpc * nk)
    )
    oma_t = np.ascontiguousarray(1.0 - alpha_t)
    return {
        "x5_t": x5_t,
        "cx_t": cx_t,
        "alpha_t": alpha_t,
        "oma_t": oma_t,
    }


def _unblock_out(ob, t_len):
    """[128, nk*192] planar (c,f) blocked -> (t, 96, 2) interleaved."""
    nk = ob.shape[1] // FC
    a = ob.reshape(128, nk, 2, F).transpose(1, 0, 2, 3).reshape(nk * 128, 2, F)
    return np.ascontiguousarray(a[:t_len].transpose(0, 2, 1))  # (t, 96, 2)


def run_on_cores(spec, coefs, alpha, trace=False):
    """Full-input entry: shard, run on 8 cores, return (out_full, results_obj)."""
    from concourse import bass_utils

    nc = _get_program()
    in_maps = [
        _host_prep(
            spec[c * BPC : (c + 1) * BPC],
            coefs[c * BPC : (c + 1) * BPC],
            alpha[c * BPC : (c + 1) * BPC],
            BPC,
            T,
        )
        for c in range(NCORES)
    ]
    res = bass_utils.run_bass_kernel_spmd(
        nc, in_maps, core_ids=list(range(NCORES)), trace=trace
    )
    full = np.array(spec, dtype=np.float32, copy=True)  # f>=96 passthrough on host
    for c in range(NCORES):
        ob = res.results[c]["outb"]
        for b in range(BPC):
            full[c * BPC + b, 0, :, :F, :] = _unblock_out(ob[b], T)
    return full, res


def kernel(spec, coefs, alpha):
    spec = np.asarray(spec, dtype=np.float32)
    coefs = np.asarray(coefs, dtype=np.float32)
    alpha = np.asarray(alpha, dtype=np.float32)
    full, _ = run_on_cores(spec, coefs, alpha, trace=False)
    return full



# revision 2
# speedup vs baseline: 2.3672x; 2.3672x over previous
"""Trainium2 Bass kernel for nn_DfOpCoefLoop (deep-filter complex FIR + alpha blend).

Reference semantics (per batch b, time t, freq bin f < 96):
    spec_f[t,f] = sum_{i=0..4} x[t+i-2, f] * coefs[t,i,f]      (complex MAC, zero-padded in t)
    out[t,f]    = alpha[t] * spec_f[t,f] + (1-alpha[t]) * x[t,f]
    out[t,f]    = spec[t,f]                                    (f >= 96 passthrough)

Key transformations vs a naive port:
  - Alpha folding (host): c' = alpha*c with (1-alpha) added to the real coef of
    the center tap (i=2), so the device computes a pure 5-tap complex FIR:
        out[t,f] = sum_i x[t+i-2, f] * c'[t,i,f]
    No blend stage, no alpha tables on device.
  - Layout: rows = (batch, freq) pairs -> 4*96 = 384 rows/core = 3 tiles of 128
    partitions; time runs along the free dim. Tap shifts become free-dim offsets
    (plain contiguous slices), so x is loaded ONCE (no 5x window inflation).
  - fp16 everywhere: DVE TensorTensor supports the 2x_1p perf mode (0.5
    cycles/elem) for 2-byte dtypes with contiguous innermost dims.
    TensorReduce/ScalarTensorTensor never get that mode, so the tap-sum is a
    small tree of TensorTensor adds instead of a tensor_reduce.

Per row-tile (128 rows, t=0..999):
    P1[k=0..4]  = xr_i * cr_i      P1[k=5..9] = xi_i * (-ci_i)
    P2[k=0..4]  = xi_i * cr_i      P2[k=5..9] = xr_i * (-ci_i)
    re = sum(P1)  (tree: [0:5]+[5:10] -> [0:2]+[2:4] -> +, + slot4)
    im = sum(P2[0:5]) - sum(P2[5:10])  (tree with leading subtract)
Coef tile is shared between P1 and P2 (k slots 0..4 = cr taps, 5..9 = -ci taps).

Pure data-parallel over batch: 32 batches -> 8 cores x 4 batches.
The f>=96 bins never touch the device (host passthrough).
"""

import numpy as np

ORDER = 5
LOOKAHEAD = 2
F = 96
T = 1000
TP = T + ORDER - 1          # 1004 padded time samples
B = 32
NCORES = 8
BPC = B // NCORES           # 4 batches per core
ROWS = BPC * F              # 384 rows per core
NRT = ROWS // 128           # 3 row-tiles per core

_CACHE = {}


def _build_program():
    import concourse.bacc as bacc
    import concourse.mybir as mybir
    import concourse.tile as tile

    nc = bacc.Bacc("TRN2", target_bir_lowering=False, debug=False)
    dt = mybir.dt.float16

    x_t = nc.dram_tensor("x_t", [NRT, 128, 2 * TP], dt, kind="ExternalInput").ap()
    cx_t = nc.dram_tensor("cx_t", [NRT, 128, 10 * T], dt, kind="ExternalInput").ap()
    out_t = nc.dram_tensor("out_t", [NRT, 128, 2 * T], dt, kind="ExternalOutput").ap()

    with tile.TileContext(nc) as tc:
        with (
            tc.tile_pool(name="xp", bufs=2) as xp,
            tc.tile_pool(name="crp", bufs=2) as crp,
            tc.tile_pool(name="cip", bufs=2) as cip,
            tc.tile_pool(name="p1p", bufs=1) as p1p,
            tc.tile_pool(name="p2p", bufs=1) as p2p,
            tc.tile_pool(name="sp", bufs=1) as sp,
            tc.tile_pool(name="ap_", bufs=1) as ap_,
            tc.tile_pool(name="tp_", bufs=1) as tp_,
            tc.tile_pool(name="obp", bufs=2) as obp,
        ):
            for rt in range(NRT):
                x = xp.tile([128, 2 * TP], dt, name="x")
                cr = crp.tile([128, 5 * T], dt, name="cr")
                ci = cip.tile([128, 5 * T], dt, name="ci")
                nc.sync.dma_start(x[:], x_t[rt])
                nc.sync.dma_start(cr[:], cx_t[rt, :, : 5 * T])
                nc.scalar.dma_start(ci[:], cx_t[rt, :, 5 * T :])

                p1 = p1p.tile([128, 10, T], dt, name="p1")
                p2 = p2p.tile([128, 10, T], dt, name="p2")
                s1 = sp.tile([128, 5, T], dt, name="s1")
                s2 = sp.tile([128, 5, T], dt, name="s2")
                a1 = ap_.tile([128, 2, T], dt, name="a1")
                a2 = ap_.tile([128, 2, T], dt, name="a2")
                t1 = tp_.tile([128, T], dt, name="t1")
                t2 = tp_.tile([128, T], dt, name="t2")
                ob = obp.tile([128, 2 * T], dt, name="ob")

                crv = cr[:].rearrange("p (i t) -> p i t", i=5, t=T)
                civ = ci[:].rearrange("p (i t) -> p i t", i=5, t=T)

                def xr(i):
                    return x[:, i : i + T]

                def xi(i):
                    return x[:, TP + i : TP + i + T]

                # cr-dependent products first (cr DMA lands before ci)
                for i in range(5):
                    nc.vector.tensor_mul(p1[:, i], xr(i), crv[:, i])
                for i in range(5):
                    nc.vector.tensor_mul(p2[:, i], xi(i), crv[:, i])
                for i in range(5):
                    nc.vector.tensor_mul(p1[:, 5 + i], xi(i), civ[:, i])
                # re tree while p2's second half is still being produced
                nc.vector.tensor_add(s1[:], p1[:, 0:5], p1[:, 5:10])
                nc.vector.tensor_add(a1[:], s1[:, 0:2], s1[:, 2:4])
                nc.vector.tensor_add(t1[:], a1[:, 0], a1[:, 1])
                nc.vector.tensor_add(ob[:, 0:T], t1[:], s1[:, 4])
                for i in range(5):
                    nc.vector.tensor_mul(p2[:, 5 + i], xr(i), civ[:, i])
                # im tree (leading subtract cancels the pre-negated ci)
                nc.vector.tensor_sub(s2[:], p2[:, 0:5], p2[:, 5:10])
                nc.vector.tensor_add(a2[:], s2[:, 0:2], s2[:, 2:4])
                nc.vector.tensor_add(t2[:], a2[:, 0], a2[:, 1])
                nc.vector.tensor_add(ob[:, T : 2 * T], t2[:], s2[:, 4])

                nc.scalar.dma_start(out_t[rt], ob[:])
    nc.compile()
    return nc


def _get_program():
    if "nc" not in _CACHE:
        _CACHE["nc"] = _build_program()
    return _CACHE["nc"]


def _host_prep(spec, coefs, alpha):
    """Build the (row, free) fp16 device layouts for all 32 batches at once."""
    x = np.asarray(spec[:, 0, :, :F, :], dtype=np.float32)      # (B, T, F, 2)
    X = np.zeros((B, F, 2, TP), np.float16)
    X[:, :, :, LOOKAHEAD : LOOKAHEAD + T] = x.transpose(0, 2, 3, 1)
    X = X.reshape(B * F, 2 * TP)

    a = np.asarray(alpha, dtype=np.float32)[:, :, 0]            # (B, T)
    cc = np.asarray(coefs, dtype=np.float32) * a[:, :, None, None, None]
    cc[:, :, LOOKAHEAD, :, 0] += 1.0 - a[:, :, None]
    # (B, T, ORDER, F, 2) -> (B, F, 2, ORDER, T), negate the ci half
    ct = cc.transpose(0, 3, 4, 2, 1)
    CX = np.empty((B, F, 2, ORDER, T), np.float16)
    CX[:, :, 0] = ct[:, :, 0]
    CX[:, :, 1] = -ct[:, :, 1]
    CX = CX.reshape(B * F, 10 * T)
    return X, CX


def run_on_cores(spec, coefs, alpha, trace=False):
    from concourse import bass_utils

    nc = _get_program()
    X, CX = _host_prep(spec, coefs, alpha)
    in_maps = []
    for c in range(NCORES):
        sl = slice(c * ROWS, (c + 1) * ROWS)
        in_maps.append(
            {
                "x_t": np.ascontiguousarray(X[sl].reshape(NRT, 128, 2 * TP)),
                "cx_t": np.ascontiguousarray(CX[sl].reshape(NRT, 128, 10 * T)),
            }
        )
    res = bass_utils.run_bass_kernel_spmd(
        nc, in_maps, core_ids=list(range(NCORES)), trace=trace
    )
    full = np.array(spec, dtype=np.float32, copy=True)  # f>=96 passthrough on host
    outs = np.concatenate(
        [res.results[c]["out_t"].reshape(ROWS, 2, T) for c in range(NCORES)]
    )                                                   # (B*F, 2, T)
    blend = outs.reshape(B, F, 2, T).transpose(0, 3, 1, 2).astype(np.float32)
    full[:, 0, :, :F, :] = blend
    return full, res


def kernel(spec, coefs, alpha):
    spec = np.asarray(spec, dtype=np.float32)
    coefs = np.asarray(coefs, dtype=np.float32)
    alpha = np.asarray(alpha, dtype=np.float32)
    full, _ = run_on_cores(spec, coefs, alpha, trace=False)
    return full


# revision 3
# speedup vs baseline: 2.6321x; 1.1119x over previous
"""Trainium2 Bass kernel for nn_DfOpCoefLoop (deep-filter complex FIR + alpha blend).

Reference semantics (per batch b, time t, freq bin f < 96):
    spec_f[t,f] = sum_{i=0..4} x[t+i-2, f] * coefs[t,i,f]      (complex MAC, zero-padded in t)
    out[t,f]    = alpha[t] * spec_f[t,f] + (1-alpha[t]) * x[t,f]
    out[t,f]    = spec[t,f]                                    (f >= 96 passthrough)

Device-side transformations:
  - Alpha folding (host): c' = alpha*c with (1-alpha) added to the real coef of
    the center tap (i=2), so the device computes a pure 5-tap complex FIR.
  - Layout: rows = (batch, freq) pairs -> 4*96 = 384 rows/core = 3 tiles of 128
    partitions; time along the free dim. Tap shifts are free-dim offsets, so x
    is loaded once (no 5x window inflation).
  - fp16 everywhere: DVE TensorTensor hits the 2x_1p mode (0.52 ns/elem).
    TensorReduce/STT never do, so all math is TensorTensor mult/add/sub.
  - Karatsuba-style 3-mult complex product per tap:
        m1_i = xr_i*cr_i   m2_i = xi_i*ci_i   m3_i = (xr_i+xi_i)*(cr_i+ci_i)
        re = R1 - R2       im = R3 - R1 - R2      (R_g = sum_i m_g_i)
    15 products instead of 20; s = xr+xi is one add shared by all taps
    (shifted views of one array); cs = cr+ci is folded on the host.
  - Products are emitted as 3 big [128, 5, 1000] TensorTensor instructions
    using hand-built overlapping access patterns (tap dim stride 1 over the
    same x row) -- verified to run at full 2x rate.
  - Tap sums via a log tree of TensorTensor adds over all 3 groups at once:
    [0:2]+[2:4] -> +  -> + slot4  (TensorReduce would be ~2x slower).

Pure data-parallel over batch: 32 batches -> 8 cores x 4 batches.
The f>=96 bins never touch the device (host passthrough).
"""

import numpy as np

ORDER = 5
LOOKAHEAD = 2
F = 96
T = 1000
TP = T + ORDER - 1          # 1004 padded time samples
B = 32
NCORES = 8
BPC = B // NCORES           # 4 batches per core
ROWS = BPC * F              # 384 rows per core
NRT = ROWS // 128           # 3 row-tiles per core

_CACHE = {}


def _build_program():
    import concourse.bacc as bacc
    import concourse.mybir as mybir
    import concourse.tile as tile
    from concourse.ap import AP

    nc = bacc.Bacc("TRN2", target_bir_lowering=False, debug=False)
    dt = mybir.dt.float16
    mul = mybir.AluOpType.mult

    x_t = nc.dram_tensor("x_t", [NRT, 128, 2 * TP], dt, kind="ExternalInput").ap()
    cx_t = nc.dram_tensor("cx_t", [NRT, 128, 15 * T], dt, kind="ExternalInput").ap()
    out_t = nc.dram_tensor("out_t", [NRT, 128, 2 * T], dt, kind="ExternalOutput").ap()

    def taps(tile_ap, row_elems, off):
        """Overlapping [128, 5, T] view: (p, i, t) -> base + off + i + t."""
        return AP(tile_ap.tensor, tile_ap.offset + off, [[row_elems, 128], [1, ORDER], [1, T]])

    with tile.TileContext(nc) as tc:
        with (
            tc.tile_pool(name="xp", bufs=2) as xp,
            tc.tile_pool(name="sp", bufs=2) as sp,
            tc.tile_pool(name="cp", bufs=2) as cp,
            tc.tile_pool(name="pp", bufs=1) as pp,
            tc.tile_pool(name="ap_", bufs=1) as ap_,
            tc.tile_pool(name="bp", bufs=1) as bp,
            tc.tile_pool(name="rp", bufs=1) as rp,
            tc.tile_pool(name="obp", bufs=2) as obp,
        ):
            for rt in range(NRT):
                x = xp.tile([128, 2 * TP], dt, name="x")
                s = sp.tile([128, TP], dt, name="s")
                c = cp.tile([128, 15, T], dt, name="c")
                nc.sync.dma_start(x[:], x_t[rt])
                nc.sync.dma_start(c[:, 0:5], cx_t[rt, :, : 5 * T])
                nc.scalar.dma_start(c[:, 5:15], cx_t[rt, :, 5 * T :])

                p = pp.tile([128, 3, ORDER, T], dt, name="p")
                a = ap_.tile([128, 3, 2, T], dt, name="a")
                bb = bp.tile([128, 3, T], dt, name="bb")
                r = rp.tile([128, 3, T], dt, name="r")
                ti = rp.tile([128, T], dt, name="ti")
                ob = obp.tile([128, 2 * T], dt, name="ob")

                # s = xr + xi (shared by all taps via shifted views)
                nc.vector.tensor_add(s[:], x[:, 0:TP], x[:, TP : 2 * TP])
                # products: one big overlapped-AP mult per group
                nc.vector.tensor_tensor(p[:, 0], taps(x[:], 2 * TP, 0), c[:, 0:5], op=mul)
                nc.vector.tensor_tensor(p[:, 1], taps(x[:], 2 * TP, TP), c[:, 5:10], op=mul)
                nc.vector.tensor_tensor(p[:, 2], taps(s[:], TP, 0), c[:, 10:15], op=mul)
                # tap-sum tree over all 3 groups at once
                nc.vector.tensor_add(a[:], p[:, :, 0:2], p[:, :, 2:4])
                nc.vector.tensor_add(bb[:], a[:, :, 0], a[:, :, 1])
                nc.vector.tensor_add(r[:], bb[:], p[:, :, 4])
                # combines: re = R1 - R2 ; im = R3 - R1 - R2
                nc.vector.tensor_sub(ob[:, 0:T], r[:, 0], r[:, 1])
                nc.vector.tensor_sub(ti[:], r[:, 2], r[:, 0])
                nc.vector.tensor_sub(ob[:, T : 2 * T], ti[:], r[:, 1])

                nc.scalar.dma_start(out_t[rt], ob[:])
    nc.compile()
    return nc


def _get_program():
    if "nc" not in _CACHE:
        _CACHE["nc"] = _build_program()
    return _CACHE["nc"]


def _host_prep(spec, coefs, alpha):
    """Build the (row, free) fp16 device layouts for all 32 batches at once."""
    x = np.asarray(spec[:, 0, :, :F, :], dtype=np.float32)      # (B, T, F, 2)
    X = np.zeros((B, F, 2, TP), np.float16)
    X[:, :, :, LOOKAHEAD : LOOKAHEAD + T] = x.transpose(0, 2, 3, 1)
    X = X.reshape(B * F, 2 * TP)

    a = np.asarray(alpha, dtype=np.float32)[:, :, 0]            # (B, T)
    cc = np.asarray(coefs, dtype=np.float32) * a[:, :, None, None, None]
    cc[:, :, LOOKAHEAD, :, 0] += 1.0 - a[:, :, None]
    # (B, T, ORDER, F, 2) -> (B, F, 2, ORDER, T)
    ct = cc.transpose(0, 3, 4, 2, 1)
    CX = np.empty((B, F, 3, ORDER, T), np.float16)
    CX[:, :, 0] = ct[:, :, 0]                                   # cr
    CX[:, :, 1] = ct[:, :, 1]                                   # ci
    CX[:, :, 2] = ct[:, :, 0] + ct[:, :, 1]                     # cs = cr + ci
    CX = CX.reshape(B * F, 15 * T)
    return X, CX


def run_on_cores(spec, coefs, alpha, trace=False):
    from concourse import bass_utils

    nc = _get_program()
    X, CX = _host_prep(spec, coefs, alpha)
    in_maps = []
    for c in range(NCORES):
        sl = slice(c * ROWS, (c + 1) * ROWS)
        in_maps.append(
            {
                "x_t": np.ascontiguousarray(X[sl].reshape(NRT, 128, 2 * TP)),
                "cx_t": np.ascontiguousarray(CX[sl].reshape(NRT, 128, 15 * T)),
            }
        )
    res = bass_utils.run_bass_kernel_spmd(
        nc, in_maps, core_ids=list(range(NCORES)), trace=trace
    )
    full = np.array(spec, dtype=np.float32, copy=True)  # f>=96 passthrough on host
    outs = np.concatenate(
        [res.results[c]["out_t"].reshape(ROWS, 2, T) for c in range(NCORES)]
    )                                                   # (B*F, 2, T)
    blend = outs.reshape(B, F, 2, T).transpose(0, 3, 1, 2).astype(np.float32)
    full[:, 0, :, :F, :] = blend
    return full, res


def kernel(spec, coefs, alpha):
    spec = np.asarray(spec, dtype=np.float32)
    coefs = np.asarray(coefs, dtype=np.float32)
    alpha = np.asarray(alpha, dtype=np.float32)
    full, _ = run_on_cores(spec, coefs, alpha, trace=False)
    return full
